# revision 1
# baseline (speedup 1.0000x reference)
"""nn_AttentionReducer kernel: data-parallel over batch on 8 NeuronCores.

Shards x/attention_mask along batch dim 0 across the 8 cores (2 batches
per core), replicates params, runs the local-window-attention encoder
forward on each core, gathers the full (16, 2048, 1024) output.
"""
import numpy as np
import jax
import jax.numpy as jnp
from functools import partial

B, N = 16, 2048
EMBED_DIM = 384
INPUT_DIM = 1024
HEADS = 8
DIM_HEAD = 48
INNER = HEADS * DIM_HEAD
WINDOW = 64
DEPTH = 3
FF_INNER = 1024
XPOS_BASE = WINDOW // 2
NEG = -1e30
EPS = 1e-5
N_CORES = 8


def _layernorm(x, g, b):
    m = jnp.mean(x, -1, keepdims=True)
    v = jnp.var(x, -1, keepdims=True)
    return (x - m) / jnp.sqrt(v + EPS) * g + b


def _rotate_half(x):
    d = x.shape[-1] // 2
    x1, x2 = x[..., :d], x[..., d:]
    return jnp.concatenate((-x2, x1), axis=-1)


def _xpos_tables(n, d, base):
    inv_freq = 1.0 / (10000.0 ** (jnp.arange(0, d, 2, dtype=jnp.float32) / d))
    t = jnp.arange(n, dtype=jnp.float32)
    freqs = t[:, None] * inv_freq[None, :]
    freqs = jnp.concatenate((freqs, freqs), -1)
    sv = (jnp.arange(0, d, 2, dtype=jnp.float32) + 0.4 * d) / (1.4 * d)
    power = (t - n // 2) / base
    scale = sv[None, :] ** power[:, None]
    scale = jnp.concatenate((scale, scale), -1)
    return freqs, scale


def _look_around(x, pad_value):
    w = x.shape[-3]
    pad_width = [(0, 0)] * (x.ndim - 3) + [(1, 1), (0, 0), (0, 0)]
    p = jnp.pad(x, pad_width, constant_values=pad_value)
    return jnp.concatenate([p[..., i:i + w, :, :] for i in range(3)], axis=-2)


def _attn_block(x, mask, p, cos, sin, scale):
    b, n, _ = x.shape
    h = _layernorm(x, p['attn_ln_g'], p['attn_ln_b'])
    qkv = h @ p['Wqkv']
    q, k, v = jnp.split(qkv, 3, axis=-1)
    to_heads = lambda t: t.reshape(b, n, HEADS, DIM_HEAD).transpose(0, 2, 1, 3)
    q, k, v = to_heads(q), to_heads(k), to_heads(v)
    q = q * cos * scale + _rotate_half(q) * sin * scale
    k = k * cos / scale + _rotate_half(k) * sin / scale
    q = q * (DIM_HEAD ** -0.5)
    w = n // WINDOW
    bq = q.reshape(b, HEADS, w, WINDOW, DIM_HEAD)
    bk = _look_around(k.reshape(b, HEADS, w, WINDOW, DIM_HEAD), 0.0)
    bv = _look_around(v.reshape(b, HEADS, w, WINDOW, DIM_HEAD), 0.0)
    sim = jnp.einsum('bhwid,bhwjd->bhwij', bq, bk)
    kpos = _look_around(
        jnp.arange(n, dtype=jnp.int32).reshape(w, WINDOW)[..., None], -1)[..., 0]
    kmask = _look_around(
        mask.astype(jnp.int32).reshape(b, w, WINDOW)[..., None], 0)[..., 0]
    valid = (kpos[None, None, :, None, :] >= 0) & (kmask[:, None, :, None, :] > 0)
    sim = jnp.where(valid, sim, NEG)
    attn = jax.nn.softmax(sim, axis=-1)
    out = jnp.einsum('bhwij,bhwjd->bhwid', attn, bv)
    out = out.reshape(b, HEADS, n, DIM_HEAD).transpose(0, 2, 1, 3).reshape(b, n, INNER)
    return out @ p['Wo']


def _ff_block(x, p):
    h = _layernorm(x, p['ff_ln_g'], p['ff_ln_b'])
    u = h @ p['W1']
    a, g = jnp.split(u, 2, axis=-1)
    return (a * jax.nn.gelu(g, approximate=False)) @ p['W2']


def _forward(x, attention_mask, params):
    rx = x[:, :, :EMBED_DIM]
    mask = attention_mask.astype(bool)
    n = rx.shape[1]
    freqs, scale = _xpos_tables(n, DIM_HEAD, XPOS_BASE)
    cos, sin = jnp.cos(freqs), jnp.sin(freqs)
    h = rx
    for lp in params['layers']:
        h = h + _attn_block(h, mask, lp, cos, sin, scale)
        h = h + _ff_block(h, lp)
    h = _layernorm(h, params['enc_ln_g'], params['enc_ln_b'])
    out = _layernorm(x + h @ params['proj_W'] + params['proj_b'],
                     params['out_ln_g'], params['out_ln_b'])
    return out


_compiled = None


def _get_compiled():
    global _compiled
    if _compiled is None:
        devices = jax.devices()[:N_CORES]
        _compiled = jax.pmap(_forward, in_axes=(0, 0, None), devices=devices)
    return _compiled


def kernel(x, attention_mask, params):
    x = np.asarray(x, dtype=np.float32)
    attention_mask = np.asarray(attention_mask, dtype=np.int32)
    params = jax.tree.map(lambda a: np.asarray(a, dtype=np.float32), params)

    # Shard dim 0 (batch) across the 8 cores: (8, 2, N, INPUT_DIM)
    per = B // N_CORES
    xs = x.reshape(N_CORES, per, N, INPUT_DIM)
    ms = attention_mask.reshape(N_CORES, per, N)

    fn = _get_compiled()
    out = fn(xs, ms, params)
    out = np.asarray(out, dtype=np.float32).reshape(B, N, INPUT_DIM)
    return out


# revision 2
# speedup vs baseline: 1.6880x; 1.6880x over previous
"""nn_AttentionReducer kernel: data-parallel over batch on 8 NeuronCores.

Shards x/attention_mask along batch dim 0 across the 8 cores (2 batches
per core), replicates params, runs the local-window-attention encoder
forward on each core, gathers the full (16, 2048, 1024) output.
"""
import numpy as np
import jax
import jax.numpy as jnp
from functools import partial

B, N = 16, 2048
EMBED_DIM = 384
INPUT_DIM = 1024
HEADS = 8
DIM_HEAD = 48
INNER = HEADS * DIM_HEAD
WINDOW = 64
DEPTH = 3
FF_INNER = 1024
XPOS_BASE = WINDOW // 2
NEG = -1e30
EPS = 1e-5
N_CORES = 8


def _layernorm(x, g, b):
    m = jnp.mean(x, -1, keepdims=True)
    v = jnp.var(x, -1, keepdims=True)
    return (x - m) / jnp.sqrt(v + EPS) * g + b


def _rotate_half(x):
    d = x.shape[-1] // 2
    x1, x2 = x[..., :d], x[..., d:]
    return jnp.concatenate((-x2, x1), axis=-1)


def _xpos_tables(n, d, base):
    inv_freq = 1.0 / (10000.0 ** (jnp.arange(0, d, 2, dtype=jnp.float32) / d))
    t = jnp.arange(n, dtype=jnp.float32)
    freqs = t[:, None] * inv_freq[None, :]
    freqs = jnp.concatenate((freqs, freqs), -1)
    sv = (jnp.arange(0, d, 2, dtype=jnp.float32) + 0.4 * d) / (1.4 * d)
    power = (t - n // 2) / base
    scale = sv[None, :] ** power[:, None]
    scale = jnp.concatenate((scale, scale), -1)
    return freqs, scale


def _look_around(x, pad_value):
    w = x.shape[-3]
    pad_width = [(0, 0)] * (x.ndim - 3) + [(1, 1), (0, 0), (0, 0)]
    p = jnp.pad(x, pad_width, constant_values=pad_value)
    return jnp.concatenate([p[..., i:i + w, :, :] for i in range(3)], axis=-2)


def _attn_block(x, mask, p, cos, sin, scale):
    b, n, _ = x.shape
    h = _layernorm(x, p['attn_ln_g'], p['attn_ln_b'])
    qkv = h @ p['Wqkv']
    q, k, v = jnp.split(qkv, 3, axis=-1)
    to_heads = lambda t: t.reshape(b, n, HEADS, DIM_HEAD).transpose(0, 2, 1, 3)
    q, k, v = to_heads(q), to_heads(k), to_heads(v)
    q = q * cos * scale + _rotate_half(q) * sin * scale
    k = k * cos / scale + _rotate_half(k) * sin / scale
    q = q * (DIM_HEAD ** -0.5)
    w = n // WINDOW
    bq = q.reshape(b, HEADS, w, WINDOW, DIM_HEAD)
    bk = _look_around(k.reshape(b, HEADS, w, WINDOW, DIM_HEAD), 0.0)
    bv = _look_around(v.reshape(b, HEADS, w, WINDOW, DIM_HEAD), 0.0)
    sim = jnp.einsum('bhwid,bhwjd->bhwij', bq, bk)
    kpos = _look_around(
        jnp.arange(n, dtype=jnp.int32).reshape(w, WINDOW)[..., None], -1)[..., 0]
    kmask = _look_around(
        mask.astype(jnp.int32).reshape(b, w, WINDOW)[..., None], 0)[..., 0]
    valid = (kpos[None, None, :, None, :] >= 0) & (kmask[:, None, :, None, :] > 0)
    sim = jnp.where(valid, sim, NEG)
    attn = jax.nn.softmax(sim, axis=-1)
    out = jnp.einsum('bhwij,bhwjd->bhwid', attn, bv)
    out = out.reshape(b, HEADS, n, DIM_HEAD).transpose(0, 2, 1, 3).reshape(b, n, INNER)
    return out @ p['Wo']


def _ff_block(x, p):
    h = _layernorm(x, p['ff_ln_g'], p['ff_ln_b'])
    u = h @ p['W1']
    a, g = jnp.split(u, 2, axis=-1)
    return (a * jax.nn.gelu(g, approximate=False)) @ p['W2']


def _forward(x, attention_mask, params):
    rx = x[:, :, :EMBED_DIM]
    mask = attention_mask.astype(bool)
    n = rx.shape[1]
    freqs, scale = _xpos_tables(n, DIM_HEAD, XPOS_BASE)
    cos, sin = jnp.cos(freqs), jnp.sin(freqs)
    h = rx
    for lp in params['layers']:
        h = h + _attn_block(h, mask, lp, cos, sin, scale)
        h = h + _ff_block(h, lp)
    h = _layernorm(h, params['enc_ln_g'], params['enc_ln_b'])
    out = _layernorm(x + h @ params['proj_W'] + params['proj_b'],
                     params['out_ln_g'], params['out_ln_b'])
    return out


_compiled = None
_param_cache = {}


def _get_compiled():
    global _compiled
    if _compiled is None:
        devices = jax.devices()[:N_CORES]
        _compiled = jax.pmap(_forward, in_axes=(0, 0, 0), devices=devices)
    return _compiled


def _params_digest(params):
    import hashlib
    h = hashlib.blake2b(digest_size=16)
    for leaf in jax.tree.leaves(params):
        a = np.ascontiguousarray(leaf)
        h.update(str(a.shape).encode())
        h.update(a.tobytes())
    return h.digest()


def _replicated_params(params):
    """Device-resident replicated params, cached across calls."""
    key = _params_digest(params)
    hit = _param_cache.get(key)
    if hit is not None:
        return hit
    devices = jax.devices()[:N_CORES]
    rep = jax.device_put_replicated(params, devices)
    _param_cache.clear()
    _param_cache[key] = rep
    return rep


def kernel(x, attention_mask, params):
    x = np.asarray(x, dtype=np.float32)
    attention_mask = np.asarray(attention_mask, dtype=np.int32)
    params = jax.tree.map(lambda a: np.asarray(a, dtype=np.float32), params)

    # Shard dim 0 (batch) across the 8 cores: (8, 2, N, INPUT_DIM)
    per = B // N_CORES
    xs = x.reshape(N_CORES, per, N, INPUT_DIM)
    ms = attention_mask.reshape(N_CORES, per, N)

    fn = _get_compiled()
    out = fn(xs, ms, _replicated_params(params))
    out = np.asarray(out, dtype=np.float32).reshape(B, N, INPUT_DIM)
    return out
